# revision 3
# baseline (speedup 1.0000x reference)
"""Multi-head self-attention Trainium2 kernel (8 NeuronCores, SPMD).

Problem: B=2, S=2048, D=1024, H=16, Dk=64; torch-style Linear projections
(x @ W.T + b), custom softmax: p = exp(scores/8), attn = p / (sum(p) + 1e-8).

Sharding: 32 (batch, head) pairs over 8 cores -> core c handles batch c//4,
heads [4*(c%4), 4*(c%4)+4). Each core projects only its 256 features of
q/k/v, attention is embarrassingly parallel over (b, h).

Per-core kernel (all matmuls in fp32r: fp32 with 11 mantissa bits, ~3x the
fp32 throughput, ~1.2e-4 rounding error):
  - inputs (host-prepped): QT = Q[b].T [1024, 2048]; WqT/WkT/WvT [1024, 256]
    (slices of W.T); biases.
  - qT/kT [256, 2048] = (W slice) @ QT + b   (transposed-space projection)
  - v     [2048, 256] = QT.T @ WvT           (normal layout; bias folded into
    the final normalize: (p@v)/denom + bv, exact because sum_t p*bv = denom*bv)
  - per head: scoresT[t, s] = kT_h.T-style matmul, two heads packed into the
    PE array via tile_position row groups (0,0)/(64,0)
  - p = exp(scoresT * 0.125) on ScalarE, PSUM -> SBUF fp32r
  - ctxT_ext [65, 512-chunk] = [v_h | 1].T @ p  accumulated over 16 t-tiles;
    row 64 = softmax denominator
  - PE-transpose 128-col blocks of ctxT_ext -> [128, 65]; DVE reciprocal of
    col 64 and scalar_tensor_tensor: out = ctx * (1/denom) + bv
Output per core: [2048, 256] fp32 -> host concatenates features per batch.
"""

import sys

sys.path.insert(0, "/opt/trn_rl_repo")

from contextlib import ExitStack

import numpy as np

import concourse.bass as bass
import concourse.tile as tile
from concourse import bacc, mybir
from concourse.bass_utils import run_bass_kernel_spmd
from concourse.masks import make_identity

F32 = mybir.dt.float32
F32R = mybir.dt.float32r

S = 2048  # sequence length
D = 1024  # d_model
J = 256  # features per core (4 heads x 64)
NKT = 8  # k-tiles of the d_model contraction
NSC = 4  # s-chunks of 512
NTT = 16  # t-tiles of 128
N_CORES = 8

_cached_nc = None
last_result = None  # BassKernelResults of the most recent run (for test.py)


def _round_fp32r(x: np.ndarray) -> np.ndarray:
    """Round fp32 to fp32r (keep 11 mantissa bits, round to nearest even)."""
    u = np.ascontiguousarray(x, dtype=np.float32).view(np.uint32)
    r = (u.astype(np.uint64) + 0x7FF + ((u >> 12) & 1)) & 0xFFFFF000
    return r.astype(np.uint32).view(np.float32)


def _build():
    nc = bacc.Bacc(None, target_bir_lowering=False)

    qt = nc.dram_tensor("qt", [D, S], F32R, kind="ExternalInput")
    wq = nc.dram_tensor("wq", [D, J], F32R, kind="ExternalInput")
    wk = nc.dram_tensor("wk", [D, J], F32R, kind="ExternalInput")
    wv = nc.dram_tensor("wv", [D, J], F32R, kind="ExternalInput")
    bq = nc.dram_tensor("bq", [J], F32, kind="ExternalInput")
    bk = nc.dram_tensor("bk", [J], F32, kind="ExternalInput")
    bv = nc.dram_tensor("bv", [J], F32, kind="ExternalInput")
    out = nc.dram_tensor("out", [S, J], F32, kind="ExternalOutput")

    with tile.TileContext(nc) as tc, ExitStack() as ctx:
        wts = ctx.enter_context(tc.tile_pool(name="wts", bufs=1))
        qkp = ctx.enter_context(tc.tile_pool(name="qkp", bufs=1))
        vxp = ctx.enter_context(tc.tile_pool(name="vxp", bufs=1))
        bp = ctx.enter_context(tc.tile_pool(name="bp", bufs=1))

        # Weights: 8 k-tiles each of [128, 256]
        wq_t, wk_t, wv_t = [], [], []
        for name, dram, lst in (("wq", wq, wq_t), ("wk", wk, wk_t), ("wv", wv, wv_t)):
            for k in range(NKT):
                t = wts.tile([128, J], F32R, name=f"{name}{k}", tag=f"{name}{k}")
                nc.sync.dma_start(t[:], dram[k * 128 : (k + 1) * 128, :])
                lst.append(t)

        # Biases: bq/bk as per-partition scalars [128, 2]; bv broadcast [128, 256]
        bq_t = bp.tile([128, 2], F32, name="bqt")
        nc.sync.dma_start(bq_t[:], bq.rearrange("(m p) -> p m", p=128))
        bk_t = bp.tile([128, 2], F32, name="bkt")
        nc.sync.dma_start(bk_t[:], bk.rearrange("(m p) -> p m", p=128))
        bv_t = bp.tile([128, J], F32, name="bvt")
        bvap = bv[:]
        bv_bcast = bass.AP(tensor=bvap.tensor, offset=bvap.offset, ap=[[0, 128], [1, J]])
        nc.sync.dma_start(bv_t[:], bv_bcast)

        ident = bp.tile([128, 128], F32, name="ident")
        make_identity(nc, ident[:])

        # Persistent projected tensors
        qT = [qkp.tile([128, S], F32R, name=f"qT{m}", tag=f"qT{m}") for m in range(2)]
        kT = [qkp.tile([128, S], F32R, name=f"kT{m}", tag=f"kT{m}") for m in range(2)]
        v_ext = []
        for t in range(NTT):
            vt = vxp.tile([128, 4, 65], F32R, name=f"vx{t}", tag=f"vx{t}")
            nc.gpsimd.memset(vt[:].bitcast(F32), 1.0)  # ones col [:, h, 64] survives
            v_ext.append(vt)

        # ---- Phase 1: projections, pipelined over s-chunks of 512 ----
        with (
            tc.tile_pool(name="qtc", bufs=2) as qtcp,
            tc.tile_pool(name="pps", bufs=1, space="PSUM") as pps,
        ):
            for sc in range(NSC):
                s0 = sc * 512
                qtc = qtcp.tile([128, NKT, 512], F32R, name="qtc", tag="qtc")
                for k in range(NKT):
                    nc.sync.dma_start(
                        qtc[:, k, :], qt[k * 128 : (k + 1) * 128, s0 : s0 + 512]
                    )
                pq = [
                    pps.tile([128, 512], F32, name=f"pq{m}", tag=f"pq{m}")
                    for m in range(2)
                ]
                pk = [
                    pps.tile([128, 512], F32, name=f"pk{m}", tag=f"pk{m}")
                    for m in range(2)
                ]
                pv = [
                    pps.tile([128, J], F32, name=f"pv{i}", tag=f"pv{i}")
                    for i in range(4)
                ]
                for k in range(NKT):
                    st, sp = (k == 0), (k == NKT - 1)
                    for m in range(2):
                        nc.tensor.matmul(
                            pq[m][:],
                            wq_t[k][:, m * 128 : (m + 1) * 128],
                            qtc[:, k, :],
                            start=st,
                            stop=sp,
                        )
                        nc.tensor.matmul(
                            pk[m][:],
                            wk_t[k][:, m * 128 : (m + 1) * 128],
                            qtc[:, k, :],
                            start=st,
                            stop=sp,
                        )
                    for i in range(4):
                        nc.tensor.matmul(
                            pv[i][:],
                            qtc[:, k, i * 128 : (i + 1) * 128],
                            wv_t[k][:],
                            start=st,
                            stop=sp,
                        )
                for m in range(2):
                    nc.vector.tensor_scalar_add(
                        qT[m][:, s0 : s0 + 512], pq[m][:], bq_t[:, m : m + 1]
                    )
                    nc.vector.tensor_scalar_add(
                        kT[m][:, s0 : s0 + 512], pk[m][:], bk_t[:, m : m + 1]
                    )
                for i in range(4):
                    t = sc * 4 + i
                    nc.vector.tensor_copy(
                        v_ext[t][:, :, 0:64],
                        pv[i][:].rearrange("p (h d) -> p h d", h=4),
                    )

        # ---- Phase 2: attention ----
        with (
            tc.tile_pool(name="aps", bufs=1, space="PSUM") as aps,
            tc.tile_pool(name="pTp", bufs=4) as pTp,
            tc.tile_pool(name="cxs", bufs=3) as cxs,
            tc.tile_pool(name="outp", bufs=2) as outp,
            tc.tile_pool(name="rp", bufs=8) as rp,
        ):
            for sc in range(NSC):
                s0 = sc * 512
                out_tiles = [
                    outp.tile([128, J], F32, name=f"ot{i}", tag=f"ot{i}")
                    for i in range(4)
                ]
                for pair in range(2):
                    hA, hB = 2 * pair, 2 * pair + 1
                    qTt, kTt = qT[pair], kT[pair]
                    ctxA = aps.tile([65, 512], F32, name="ctxA", tag="ctx", bufs=3)
                    ctxB = aps.tile([65, 512], F32, name="ctxB", tag="ctx", bufs=3)
                    grps = {}
                    pts = {}
                    # software pipeline: scores(t)+exp(t), AV(t-1)
                    for t in range(NTT + 1):
                        if t < NTT:
                            tsl = slice(t * 128, (t + 1) * 128)
                            gA = aps.tile(
                                [128, 512], F32, name="gA", tag="grp", bufs=4
                            )
                            nc.tensor.matmul(
                                gA[:],
                                kTt[0:64, tsl],
                                qTt[0:64, s0 : s0 + 512],
                                start=True,
                                stop=True,
                                tile_position=(0, 0),
                            )
                            gB = aps.tile(
                                [128, 512], F32, name="gB", tag="grp", bufs=4
                            )
                            nc.tensor.matmul(
                                gB[:],
                                kTt[64:128, tsl],
                                qTt[64:128, s0 : s0 + 512],
                                start=True,
                                stop=True,
                                tile_position=(64, 0),
                            )
                            pA = pTp.tile([128, 512], F32R, name="pA", tag="pT")
                            nc.scalar.activation(
                                pA[:], gA[:], mybir.ActivationFunctionType.Exp,
                                scale=0.125,
                            )
                            pB = pTp.tile([128, 512], F32R, name="pB", tag="pT")
                            nc.scalar.activation(
                                pB[:], gB[:], mybir.ActivationFunctionType.Exp,
                                scale=0.125,
                            )
                            pts[t] = (pA, pB)
                        if t >= 1:
                            tp_ = t - 1
                            pA, pB = pts.pop(tp_)
                            st, sp = (tp_ == 0), (tp_ == NTT - 1)
                            nc.tensor.matmul(
                                ctxA[:], v_ext[tp_][:, hA, :], pA[:],
                                start=st, stop=sp,
                            )
                            nc.tensor.matmul(
                                ctxB[:], v_ext[tp_][:, hB, :], pB[:],
                                start=st, stop=sp,
                            )
                    # finalize pair: copy ctxT to SBUF, transpose, normalize
                    for h, ctx_ps in ((hA, ctxA), (hB, ctxB)):
                        cs = cxs.tile([65, 512], F32, name="cs", tag="cs")
                        nc.vector.tensor_copy(cs[:], ctx_ps[:])
                        for i in range(4):
                            tp = aps.tile(
                                [128, 65], F32, name="tp", tag="tp", bufs=1
                            )
                            nc.tensor.transpose(
                                tp[:],
                                cs[0:65, i * 128 : (i + 1) * 128],
                                ident[0:65, 0:65],
                            )
                            r = rp.tile([128, 1], F32, name="r", tag="r")
                            nc.vector.reciprocal(r[:], tp[:, 64:65])
                            nc.vector.scalar_tensor_tensor(
                                out=out_tiles[i][:, h * 64 : (h + 1) * 64],
                                in0=tp[:, 0:64],
                                scalar=r[:],
                                in1=bv_t[:, h * 64 : (h + 1) * 64],
                                op0=mybir.AluOpType.mult,
                                op1=mybir.AluOpType.add,
                            )
                for i in range(4):
                    nc.sync.dma_start(
                        out[s0 + i * 128 : s0 + (i + 1) * 128, :], out_tiles[i][:]
                    )

    nc.compile()
    return nc


def kernel(Q, Wq, bq, Wk, bk, Wv, bv):
    global _cached_nc, last_result
    Q = np.asarray(Q, dtype=np.float32)
    Wq, Wk, Wv = (np.asarray(w, dtype=np.float32) for w in (Wq, Wk, Wv))
    bq, bk, bv = (np.asarray(b, dtype=np.float32) for b in (bq, bk, bv))
    B = Q.shape[0]
    assert Q.shape == (B, S, D) and B * 4 == N_CORES

    if _cached_nc is None:
        _cached_nc = _build()
    nc = _cached_nc

    # host-side shard prep
    qts = [_round_fp32r(Q[b].T) for b in range(B)]
    wqs = [_round_fp32r(Wq[g * J : (g + 1) * J, :].T) for g in range(4)]
    wks = [_round_fp32r(Wk[g * J : (g + 1) * J, :].T) for g in range(4)]
    wvs = [_round_fp32r(Wv[g * J : (g + 1) * J, :].T) for g in range(4)]

    in_maps = []
    for c in range(N_CORES):
        b, g = c // 4, c % 4
        jsl = slice(g * J, (g + 1) * J)
        in_maps.append(
            {
                "qt": qts[b],
                "wq": wqs[g],
                "wk": wks[g],
                "wv": wvs[g],
                "bq": np.ascontiguousarray(bq[jsl]),
                "bk": np.ascontiguousarray(bk[jsl]),
                "bv": np.ascontiguousarray(bv[jsl]),
            }
        )

    last_result = run_bass_kernel_spmd(nc, in_maps, list(range(N_CORES)))

    full = np.empty((B, S, D), dtype=np.float32)
    for c in range(N_CORES):
        b, g = c // 4, c % 4
        full[b, :, g * J : (g + 1) * J] = last_result.results[c]["out"]
    return full


# revision 4
# speedup vs baseline: 1.3757x; 1.3757x over previous
"""Multi-head self-attention Trainium2 kernel (8 NeuronCores, SPMD).

Problem: B=2, S=2048, D=1024, H=16, Dk=64; torch-style Linear projections
(x @ W.T + b), custom softmax: p = exp(scores/8), attn = p / (sum(p) + 1e-8).

Sharding: 32 (batch, head) pairs over 8 cores -> core c handles batch c//4,
heads [4*(c%4), 4*(c%4)+4). Each core projects only its 256 features of
q/k/v; attention is embarrassingly parallel over (b, h).

Per-core kernel (all matmuls in fp32r: fp32 with 11 mantissa bits, ~3x the
fp32 PE throughput, ~1.2e-4 rounding error):
  - inputs (host-prepped): QT = Q[b].T [1024, 2048]; WqT/WkT/WvT [1024, 256]
    (slices of W.T); biases.
  - qT/kT [256, 2048] = (W slice) @ QT + b   (transposed-space projection;
    bias added as a per-partition scalar during the PSUM->SBUF copy)
  - v     [2048, 256] = QT.T @ WvT           (normal layout; bias folded into
    the final normalize: (p@v)/denom + bv, exact because sum_t p*bv = denom*bv)
  - per head pair: scoresT[t, s] two heads packed into the PE array via
    tile_position row groups (0,0)/(64,0); p = exp(scoresT/8) on ScalarE
  - ctxT_ext [65, 512-chunk] = [v_h | 1].T @ p accumulated over 16 t-tiles;
    row 64 = softmax denominator
  - final phase: PE-transpose 128-col blocks -> [128, 65]; DVE reciprocal of
    col 64 and scalar_tensor_tensor: out = ctx * (1/denom) + bv
HAM note: dummy keep-warm matmuls bridge the proj->attention PSUM pool
transition; a >3.4us PE idle there would drop the PE clock to 1.2GHz for the
whole attention phase (observed: 2x slowdown).
Output per core: [2048, 256] fp32 -> host concatenates features per batch.
"""

import sys

sys.path.insert(0, "/opt/trn_rl_repo")

from contextlib import ExitStack

import numpy as np

import concourse.bass as bass
import concourse.tile as tile
from concourse import bacc, mybir
from concourse.bass_utils import run_bass_kernel_spmd
from concourse.masks import make_identity

F32 = mybir.dt.float32
F32R = mybir.dt.float32r

S = 2048  # sequence length
D = 1024  # d_model
J = 256  # features per core (4 heads x 64)
NKT = 8  # k-tiles of the d_model contraction
NSC = 4  # s-chunks of 512
NTT = 16  # t-tiles of 128
N_CORES = 8
N_WARM = 24  # keep-warm dummy matmuls at the proj->attention transition

_cached_nc = None
last_result = None  # BassKernelResults of the most recent run (for test.py)


def _round_fp32r(x: np.ndarray) -> np.ndarray:
    """Round fp32 to fp32r (keep 11 mantissa bits, round to nearest even)."""
    u = np.ascontiguousarray(x, dtype=np.float32).view(np.uint32)
    r = (u.astype(np.uint64) + 0x7FF + ((u >> 12) & 1)) & 0xFFFFF000
    return r.astype(np.uint32).view(np.float32)


def _build():
    nc = bacc.Bacc(None, target_bir_lowering=False)

    qt = nc.dram_tensor("qt", [D, S], F32R, kind="ExternalInput")
    wq = nc.dram_tensor("wq", [D, J], F32R, kind="ExternalInput")
    wk = nc.dram_tensor("wk", [D, J], F32R, kind="ExternalInput")
    wv = nc.dram_tensor("wv", [D, J], F32R, kind="ExternalInput")
    bq = nc.dram_tensor("bq", [J], F32, kind="ExternalInput")
    bk = nc.dram_tensor("bk", [J], F32, kind="ExternalInput")
    bv = nc.dram_tensor("bv", [J], F32, kind="ExternalInput")
    out = nc.dram_tensor("out", [S, J], F32, kind="ExternalOutput")

    with tile.TileContext(nc) as tc, ExitStack() as ctx:
        warm = ctx.enter_context(tc.tile_pool(name="warm", bufs=1, space="PSUM"))
        wts = ctx.enter_context(tc.tile_pool(name="wts", bufs=1))
        qkp = ctx.enter_context(tc.tile_pool(name="qkp", bufs=1))
        vxp = ctx.enter_context(tc.tile_pool(name="vxp", bufs=1))
        bp = ctx.enter_context(tc.tile_pool(name="bp", bufs=1))
        cxp = ctx.enter_context(tc.tile_pool(name="cxp", bufs=1))

        warm_t = warm.tile([128, 512], F32, name="warmt", tag="warm")

        # Weights: 8 k-tiles each of [128, 256]
        wq_t, wk_t, wv_t = [], [], []
        for name, dram, lst in (("wq", wq, wq_t), ("wk", wk, wk_t), ("wv", wv, wv_t)):
            for k in range(NKT):
                t = wts.tile([128, J], F32R, name=f"{name}{k}", tag=f"{name}{k}")
                nc.sync.dma_start(t[:], dram[k * 128 : (k + 1) * 128, :])
                lst.append(t)

        # Biases: bq/bk as per-partition scalars [128, 2]; bv broadcast [128, 256]
        bq_t = bp.tile([128, 2], F32, name="bqt")
        nc.sync.dma_start(bq_t[:], bq.rearrange("(m p) -> p m", p=128))
        bk_t = bp.tile([128, 2], F32, name="bkt")
        nc.sync.dma_start(bk_t[:], bk.rearrange("(m p) -> p m", p=128))
        bv_t = bp.tile([128, J], F32, name="bvt")
        bvap = bv[:]
        bv_bcast = bass.AP(
            tensor=bvap.tensor, offset=bvap.offset, ap=[[0, 128], [1, J]]
        )
        nc.sync.dma_start(bv_t[:], bv_bcast)

        ident = bp.tile([128, 128], F32, name="ident")
        make_identity(nc, ident[:])
        scratch = bp.tile([128, 1], F32, name="scratch")

        # Persistent projected tensors
        qT = [qkp.tile([128, S], F32R, name=f"qT{m}", tag=f"qT{m}") for m in range(2)]
        kT = [qkp.tile([128, S], F32R, name=f"kT{m}", tag=f"kT{m}") for m in range(2)]
        v_ext = []
        for t in range(NTT):
            vt = vxp.tile([128, 4, 65], F32R, name=f"vx{t}", tag=f"vx{t}")
            nc.gpsimd.memset(vt[:].bitcast(F32), 1.0)  # ones col [:, h, 64] survives
            v_ext.append(vt)
        # ctxT_ext staging for the final phase, per (s-chunk, head)
        ctxs = [
            [cxp.tile([65, 512], F32, name=f"cx{sc}_{h}", tag=f"cx{sc}_{h}")
             for h in range(4)]
            for sc in range(NSC)
        ]

        # ---- Phase 1: projections, pipelined over s-chunks of 512 ----
        with (
            tc.tile_pool(name="qtc", bufs=2) as qtcp,
            tc.tile_pool(name="pps", bufs=1, space="PSUM") as pps,
        ):
            for sc in range(NSC):
                s0 = sc * 512
                qtc = qtcp.tile([128, NKT, 512], F32R, name="qtc", tag="qtc")
                for k in range(NKT):
                    nc.sync.dma_start(
                        qtc[:, k, :], qt[k * 128 : (k + 1) * 128, s0 : s0 + 512]
                    )
                pq = [
                    pps.tile([128, 512], F32, name=f"pq{m}", tag=f"pq{m}")
                    for m in range(2)
                ]
                pk = [
                    pps.tile([128, 512], F32, name=f"pk{m}", tag=f"pk{m}")
                    for m in range(2)
                ]
                # v: two sequential passes of 2 s-subtiles (saves 2 PSUM banks)
                for vpass in range(2):
                    pv = [
                        pps.tile([128, J], F32, name=f"pv{i}", tag=f"pv{i}")
                        for i in range(2)
                    ]
                    for k in range(NKT):
                        st, sp = (k == 0), (k == NKT - 1)
                        if vpass == 0:
                            for m in range(2):
                                nc.tensor.matmul(
                                    pq[m][:],
                                    wq_t[k][:, m * 128 : (m + 1) * 128],
                                    qtc[:, k, :],
                                    start=st,
                                    stop=sp,
                                )
                                nc.tensor.matmul(
                                    pk[m][:],
                                    wk_t[k][:, m * 128 : (m + 1) * 128],
                                    qtc[:, k, :],
                                    start=st,
                                    stop=sp,
                                )
                        for i in range(2):
                            sub = vpass * 2 + i
                            nc.tensor.matmul(
                                pv[i][:],
                                qtc[:, k, sub * 128 : (sub + 1) * 128],
                                wv_t[k][:],
                                start=st,
                                stop=sp,
                            )
                    if vpass == 0:
                        for m in range(2):
                            nc.vector.tensor_scalar_add(
                                qT[m][:, s0 : s0 + 512], pq[m][:], bq_t[:, m : m + 1]
                            )
                            nc.vector.tensor_scalar_add(
                                kT[m][:, s0 : s0 + 512], pk[m][:], bk_t[:, m : m + 1]
                            )
                    for i in range(2):
                        t = sc * 4 + vpass * 2 + i
                        nc.vector.tensor_copy(
                            v_ext[t][:, :, 0:64],
                            pv[i][:].rearrange("p (h d) -> p h d", h=4),
                        )
                if sc == 0:
                    # pre-load the ACT exp table set during projections so the
                    # first attention exp doesn't stall the pipeline ~2.7us
                    nc.scalar.activation(
                        scratch[:], bq_t[:, 0:1],
                        mybir.ActivationFunctionType.Exp, scale=0.0,
                    )

        # keep the PE warm across the PSUM pool transition (released-zone
        # wait is ~4us; >3.4us idle would re-throttle the PE clock)
        for w in range(N_WARM):
            nc.tensor.matmul(
                warm_t[:, 0:J],
                wv_t[0][:, 0:128],
                wv_t[1][:],
                start=True,
                stop=True,
            )

        # ---- Phase 2: attention (scores + exp + AV), PE/ACT balanced ----
        with (
            tc.tile_pool(name="aps", bufs=1, space="PSUM") as aps,
            tc.tile_pool(name="pTp", bufs=4) as pTp,
        ):
            NG = NTT // 2  # 8 groups of 2 t-tiles
            for sc in range(NSC):
                s0 = sc * 512
                for pair in range(2):
                    hA, hB = 2 * pair, 2 * pair + 1
                    qTt, kTt = qT[pair], kT[pair]
                    ctxA = aps.tile([65, 512], F32, name="ctxA", tag="ctx", bufs=2)
                    ctxB = aps.tile([65, 512], F32, name="ctxB", tag="ctx", bufs=2)
                    pts = {}
                    for g in range(NG + 1):
                        if g < NG:
                            gA = aps.tile(
                                [128, 1024], F32, name="gA", tag="grp", bufs=2
                            )
                            gB = aps.tile(
                                [128, 1024], F32, name="gB", tag="grp", bufs=2
                            )
                            for half in range(2):
                                t = 2 * g + half
                                tsl = slice(t * 128, (t + 1) * 128)
                                csl = slice(half * 512, (half + 1) * 512)
                                nc.tensor.matmul(
                                    gA[:, csl],
                                    kTt[0:64, tsl],
                                    qTt[0:64, s0 : s0 + 512],
                                    start=True,
                                    stop=True,
                                    tile_position=(0, 0),
                                )
                                nc.tensor.matmul(
                                    gB[:, csl],
                                    kTt[64:128, tsl],
                                    qTt[64:128, s0 : s0 + 512],
                                    start=True,
                                    stop=True,
                                    tile_position=(64, 0),
                                )
                            pA = pTp.tile([128, 1024], F32R, name="pA", tag="pT")
                            nc.scalar.activation(
                                pA[:], gA[:],
                                mybir.ActivationFunctionType.Exp, scale=0.125,
                            )
                            pB = pTp.tile([128, 1024], F32R, name="pB", tag="pT")
                            nc.scalar.activation(
                                pB[:], gB[:],
                                mybir.ActivationFunctionType.Exp, scale=0.125,
                            )
                            pts[g] = (pA, pB)
                        if g >= 1:
                            pA, pB = pts.pop(g - 1)
                            for half in range(2):
                                t = 2 * (g - 1) + half
                                csl = slice(half * 512, (half + 1) * 512)
                                st, sp = (t == 0), (t == NTT - 1)
                                nc.tensor.matmul(
                                    ctxA[:], v_ext[t][:, hA, :], pA[:, csl],
                                    start=st, stop=sp,
                                )
                                nc.tensor.matmul(
                                    ctxB[:], v_ext[t][:, hB, :], pB[:, csl],
                                    start=st, stop=sp,
                                )
                    nc.vector.tensor_copy(ctxs[sc][hA][:], ctxA[:])
                    nc.vector.tensor_copy(ctxs[sc][hB][:], ctxB[:])

        # ---- Phase 3: transpose + normalize + bias + store ----
        with (
            tc.tile_pool(name="tps", bufs=4, space="PSUM") as tps,
            tc.tile_pool(name="outp", bufs=2) as outp,
            tc.tile_pool(name="rp", bufs=8) as rp,
        ):
            for sc in range(NSC):
                s0 = sc * 512
                for i in range(4):
                    ot = outp.tile([128, J], F32, name="ot", tag=f"ot{i % 2}")
                    for h in range(4):
                        tp = tps.tile([128, 65], F32, name="tp", tag="tp")
                        nc.tensor.transpose(
                            tp[:],
                            ctxs[sc][h][0:65, i * 128 : (i + 1) * 128],
                            ident[0:65, 0:65],
                        )
                        r = rp.tile([128, 1], F32, name="r", tag="r")
                        nc.vector.reciprocal(r[:], tp[:, 64:65])
                        nc.vector.scalar_tensor_tensor(
                            out=ot[:, h * 64 : (h + 1) * 64],
                            in0=tp[:, 0:64],
                            scalar=r[:],
                            in1=bv_t[:, h * 64 : (h + 1) * 64],
                            op0=mybir.AluOpType.mult,
                            op1=mybir.AluOpType.add,
                        )
                    nc.sync.dma_start(
                        out[s0 + i * 128 : s0 + (i + 1) * 128, :], ot[:]
                    )

    nc.compile()
    return nc


def kernel(Q, Wq, bq, Wk, bk, Wv, bv):
    global _cached_nc, last_result
    Q = np.asarray(Q, dtype=np.float32)
    Wq, Wk, Wv = (np.asarray(w, dtype=np.float32) for w in (Wq, Wk, Wv))
    bq, bk, bv = (np.asarray(b, dtype=np.float32) for b in (bq, bk, bv))
    B = Q.shape[0]
    assert Q.shape == (B, S, D) and B * 4 == N_CORES

    if _cached_nc is None:
        _cached_nc = _build()
    nc = _cached_nc

    # host-side shard prep
    qts = [_round_fp32r(Q[b].T) for b in range(B)]
    wqs = [_round_fp32r(Wq[g * J : (g + 1) * J, :].T) for g in range(4)]
    wks = [_round_fp32r(Wk[g * J : (g + 1) * J, :].T) for g in range(4)]
    wvs = [_round_fp32r(Wv[g * J : (g + 1) * J, :].T) for g in range(4)]

    in_maps = []
    for c in range(N_CORES):
        b, g = c // 4, c % 4
        jsl = slice(g * J, (g + 1) * J)
        in_maps.append(
            {
                "qt": qts[b],
                "wq": wqs[g],
                "wk": wks[g],
                "wv": wvs[g],
                "bq": np.ascontiguousarray(bq[jsl]),
                "bk": np.ascontiguousarray(bk[jsl]),
                "bv": np.ascontiguousarray(bv[jsl]),
            }
        )

    last_result = run_bass_kernel_spmd(nc, in_maps, list(range(N_CORES)))

    full = np.empty((B, S, D), dtype=np.float32)
    for c in range(N_CORES):
        b, g = c // 4, c % 4
        full[b, :, g * J : (g + 1) * J] = last_result.results[c]["out"]
    return full


# revision 6
# speedup vs baseline: 1.5841x; 1.1514x over previous
"""Multi-head self-attention Trainium2 kernel (8 NeuronCores, SPMD).

Problem: B=2, S=2048, D=1024, H=16, Dk=64; torch-style Linear projections
(x @ W.T + b), custom softmax: p = exp(scores/8), attn = p / (sum(p) + 1e-8).

Sharding: 32 (batch, head) pairs over 8 cores -> core c handles batch c//4,
heads [4*(c%4), 4*(c%4)+4). Each core projects only its 256 features of
q/k/v; attention is embarrassingly parallel over (b, h).

Per-core kernel (all matmuls in fp32r: fp32 with 11 mantissa bits, ~3x the
fp32 PE throughput, ~1.2e-4 rounding error):
  - inputs (host-prepped): QT = Q[b].T [1024, 2048]; WqT/WkT/WvT [1024, 256]
    (slices of W.T); biases.
  - qT/kT [256, 2048] = (W slice) @ QT + b   (transposed-space projection;
    bias added as a per-partition scalar during the PSUM->SBUF copy)
  - v     [2048, 256] = QT.T @ WvT           (normal layout; bias folded into
    the final normalize: (p@v)/denom + bv, exact because sum_t p*bv = denom*bv)
  - per head pair: scoresT[t, s] two heads packed into the PE array via
    tile_position row groups (0,0)/(64,0); p = exp(scoresT/8) on ScalarE
  - ctxT_ext [65, 512-chunk] = [v_h | 1].T @ p accumulated over 16 t-tiles;
    row 64 = softmax denominator
  - final phase: PE-transpose 128-col blocks -> [128, 65]; DVE reciprocal of
    col 64 and scalar_tensor_tensor: out = ctx * (1/denom) + bv
HAM note: dummy keep-warm matmuls bridge the proj->attention PSUM pool
transition; a >3.4us PE idle there would drop the PE clock to 1.2GHz for the
whole attention phase (observed: 2x slowdown).
Output per core: [2048, 256] fp32 -> host concatenates features per batch.
"""

import sys

sys.path.insert(0, "/opt/trn_rl_repo")

from contextlib import ExitStack

import numpy as np

import concourse.bass as bass
import concourse.tile as tile
from concourse import bacc, mybir
from concourse.bass_utils import run_bass_kernel_spmd
from concourse.masks import make_identity

F32 = mybir.dt.float32
F32R = mybir.dt.float32r

S = 2048  # sequence length
D = 1024  # d_model
J = 256  # features per core (4 heads x 64)
NKT = 8  # k-tiles of the d_model contraction
NSC = 4  # s-chunks of 512
NTT = 16  # t-tiles of 128
N_CORES = 8
N_WARM = 24  # keep-warm dummy matmuls at the proj->attention transition

_cached_nc = None
last_result = None  # BassKernelResults of the most recent run (for test.py)


def _round_fp32r(x: np.ndarray) -> np.ndarray:
    """Round fp32 to fp32r (keep 11 mantissa bits, round to nearest even)."""
    u = np.ascontiguousarray(x, dtype=np.float32).view(np.uint32)
    r = (u.astype(np.uint64) + 0x7FF + ((u >> 12) & 1)) & 0xFFFFF000
    return r.astype(np.uint32).view(np.float32)


def _build():
    nc = bacc.Bacc(None, target_bir_lowering=False)

    qt = nc.dram_tensor("qt", [D, S], F32R, kind="ExternalInput")
    wq = nc.dram_tensor("wq", [D, J], F32R, kind="ExternalInput")
    wk = nc.dram_tensor("wk", [D, J], F32R, kind="ExternalInput")
    wv = nc.dram_tensor("wv", [D, J], F32R, kind="ExternalInput")
    bq = nc.dram_tensor("bq", [J], F32, kind="ExternalInput")
    bk = nc.dram_tensor("bk", [J], F32, kind="ExternalInput")
    bv = nc.dram_tensor("bv", [J], F32, kind="ExternalInput")
    out = nc.dram_tensor("out", [S, J], F32, kind="ExternalOutput")

    with tile.TileContext(nc) as tc, ExitStack() as ctx:
        warm = ctx.enter_context(tc.tile_pool(name="warm", bufs=1, space="PSUM"))
        wts = ctx.enter_context(tc.tile_pool(name="wts", bufs=1))
        qkp = ctx.enter_context(tc.tile_pool(name="qkp", bufs=1))
        vxp = ctx.enter_context(tc.tile_pool(name="vxp", bufs=1))
        bp = ctx.enter_context(tc.tile_pool(name="bp", bufs=1))
        cxp = ctx.enter_context(tc.tile_pool(name="cxp", bufs=1))

        warm_t = warm.tile([128, 512], F32, name="warmt", tag="warm")

        # Weights: 8 k-tiles each of [128, 256]
        wq_t, wk_t, wv_t = [], [], []
        for name, dram, lst in (("wq", wq, wq_t), ("wk", wk, wk_t), ("wv", wv, wv_t)):
            for k in range(NKT):
                t = wts.tile([128, J], F32R, name=f"{name}{k}", tag=f"{name}{k}")
                nc.sync.dma_start(t[:], dram[k * 128 : (k + 1) * 128, :])
                lst.append(t)

        # Biases: bq/bk as per-partition scalars [128, 2]; bv broadcast [128, 256]
        bq_t = bp.tile([128, 2], F32, name="bqt")
        nc.sync.dma_start(bq_t[:], bq.rearrange("(m p) -> p m", p=128))
        bk_t = bp.tile([128, 2], F32, name="bkt")
        nc.sync.dma_start(bk_t[:], bk.rearrange("(m p) -> p m", p=128))
        bv_t = bp.tile([128, J], F32, name="bvt")
        bvap = bv[:]
        bv_bcast = bass.AP(
            tensor=bvap.tensor, offset=bvap.offset, ap=[[0, 128], [1, J]]
        )
        nc.sync.dma_start(bv_t[:], bv_bcast)

        ident = bp.tile([128, 128], F32, name="ident")
        make_identity(nc, ident[:])
        scratch = bp.tile([128, 1], F32, name="scratch")

        # Persistent projected tensors
        qT = [qkp.tile([128, S], F32R, name=f"qT{m}", tag=f"qT{m}") for m in range(2)]
        kT = [qkp.tile([128, S], F32R, name=f"kT{m}", tag=f"kT{m}") for m in range(2)]
        v_ext = []
        for t in range(NTT):
            vt = vxp.tile([128, 4, 65], F32R, name=f"vx{t}", tag=f"vx{t}")
            nc.gpsimd.memset(vt[:].bitcast(F32), 1.0)  # ones col [:, h, 64] survives
            v_ext.append(vt)
        # ctxT_ext staging for the final phase, per (s-chunk, head)
        ctxs = [
            [cxp.tile([65, 512], F32, name=f"cx{sc}_{h}", tag=f"cx{sc}_{h}")
             for h in range(4)]
            for sc in range(NSC)
        ]

        # ---- Phase 1: projections, pipelined over s-chunks of 512 ----
        with (
            tc.tile_pool(name="qtc", bufs=3) as qtcp,
            tc.tile_pool(name="pps", bufs=1, space="PSUM") as pps,
        ):
            for sc in range(NSC):
                s0 = sc * 512
                qtc = qtcp.tile([128, NKT, 512], F32R, name="qtc", tag="qtc")
                for k in range(NKT):
                    eng = nc.sync if k % 2 == 0 else nc.gpsimd
                    eng.dma_start(
                        qtc[:, k, :], qt[k * 128 : (k + 1) * 128, s0 : s0 + 512]
                    )
                pq = [
                    pps.tile([128, 512], F32, name=f"pq{m}", tag=f"pq{m}")
                    for m in range(2)
                ]
                pk = [
                    pps.tile([128, 512], F32, name=f"pk{m}", tag=f"pk{m}")
                    for m in range(2)
                ]
                # v: two sequential passes of 2 s-subtiles (saves 2 PSUM banks)
                for vpass in range(2):
                    pv = [
                        pps.tile([128, J], F32, name=f"pv{i}", tag=f"pv{i}")
                        for i in range(2)
                    ]
                    for k in range(NKT):
                        st, sp = (k == 0), (k == NKT - 1)
                        if vpass == 0:
                            for m in range(2):
                                nc.tensor.matmul(
                                    pq[m][:],
                                    wq_t[k][:, m * 128 : (m + 1) * 128],
                                    qtc[:, k, :],
                                    start=st,
                                    stop=sp,
                                )
                                nc.tensor.matmul(
                                    pk[m][:],
                                    wk_t[k][:, m * 128 : (m + 1) * 128],
                                    qtc[:, k, :],
                                    start=st,
                                    stop=sp,
                                )
                        for i in range(2):
                            sub = vpass * 2 + i
                            nc.tensor.matmul(
                                pv[i][:],
                                qtc[:, k, sub * 128 : (sub + 1) * 128],
                                wv_t[k][:],
                                start=st,
                                stop=sp,
                            )
                    if vpass == 0:
                        for m in range(2):
                            nc.vector.tensor_scalar_add(
                                qT[m][:, s0 : s0 + 512], pq[m][:], bq_t[:, m : m + 1]
                            )
                            nc.vector.tensor_scalar_add(
                                kT[m][:, s0 : s0 + 512], pk[m][:], bk_t[:, m : m + 1]
                            )
                    for i in range(2):
                        t = sc * 4 + vpass * 2 + i
                        nc.vector.tensor_copy(
                            v_ext[t][:, :, 0:64],
                            pv[i][:].rearrange("p (h d) -> p h d", h=4),
                        )
                if sc == 0:
                    # pre-load the ACT exp table set during projections so the
                    # first attention exp doesn't stall the pipeline ~2.7us
                    nc.scalar.activation(
                        scratch[:], bq_t[:, 0:1],
                        mybir.ActivationFunctionType.Exp, scale=0.0,
                    )

        # keep the PE warm across the PSUM pool transition (released-zone
        # wait is ~4us; >3.4us idle would re-throttle the PE clock)
        for w in range(N_WARM):
            nc.tensor.matmul(
                warm_t[:, 0:J],
                wv_t[0][:, 0:128],
                wv_t[1][:],
                start=True,
                stop=True,
            )

        # ---- Phase 2: attention (scores + exp + AV) ----
        # ACT is the natural bottleneck (1140ns/t vs PE ~920ns/t); a dummy
        # matmul per t-step keeps the PE strictly busier than ACT so the HAM
        # clock gate never sees PE idle and the PE stays at 2.4GHz.
        with (
            tc.tile_pool(name="aps", bufs=1, space="PSUM") as aps,
            tc.tile_pool(name="pTp", bufs=4) as pTp,
        ):
            for sc in range(NSC):
                s0 = sc * 512
                for pair in range(2):
                    hA, hB = 2 * pair, 2 * pair + 1
                    qTt, kTt = qT[pair], kT[pair]
                    ctxA = aps.tile([65, 512], F32, name="ctxA", tag="ctx", bufs=2)
                    ctxB = aps.tile([65, 512], F32, name="ctxB", tag="ctx", bufs=2)
                    pts = {}
                    for t in range(NTT + 1):
                        if t < NTT:
                            tsl = slice(t * 128, (t + 1) * 128)
                            gA = aps.tile(
                                [128, 512], F32, name="gA", tag="grp", bufs=4
                            )
                            gB = aps.tile(
                                [128, 512], F32, name="gB", tag="grp", bufs=4
                            )
                            nc.tensor.matmul(
                                gA[:],
                                kTt[0:64, tsl],
                                qTt[0:64, s0 : s0 + 512],
                                start=True,
                                stop=True,
                                tile_position=(0, 0),
                            )
                            nc.tensor.matmul(
                                gB[:],
                                kTt[64:128, tsl],
                                qTt[64:128, s0 : s0 + 512],
                                start=True,
                                stop=True,
                                tile_position=(64, 0),
                            )
                            pA = pTp.tile([128, 512], F32R, name="pA", tag="pT")
                            nc.scalar.activation(
                                pA[:], gA[:],
                                mybir.ActivationFunctionType.Exp, scale=0.125,
                            )
                            pB = pTp.tile([128, 512], F32R, name="pB", tag="pT")
                            nc.scalar.activation(
                                pB[:], gB[:],
                                mybir.ActivationFunctionType.Exp, scale=0.125,
                            )
                            pts[t] = (pA, pB)
                        if t >= 1:
                            pA, pB = pts.pop(t - 1)
                            st, sp = (t - 1 == 0), (t - 1 == NTT - 1)
                            nc.tensor.matmul(
                                ctxA[:], v_ext[t - 1][:, hA, :], pA[:],
                                start=st, stop=sp,
                            )
                            nc.tensor.matmul(
                                ctxB[:], v_ext[t - 1][:, hB, :], pB[:],
                                start=st, stop=sp,
                            )
                        # HAM keep-warm filler
                        nc.tensor.matmul(
                            warm_t[:, 0:J],
                            wv_t[0][:, 0:128],
                            wv_t[1][:],
                            start=True,
                            stop=True,
                        )
                    nc.vector.tensor_copy(ctxs[sc][hA][:], ctxA[:])
                    nc.vector.tensor_copy(ctxs[sc][hB][:], ctxB[:])

        # ---- Phase 3: transpose + normalize + bias + store ----
        with (
            tc.tile_pool(name="tps", bufs=4, space="PSUM") as tps,
            tc.tile_pool(name="outp", bufs=2) as outp,
            tc.tile_pool(name="rp", bufs=8) as rp,
        ):
            for sc in range(NSC):
                s0 = sc * 512
                for i in range(4):
                    ot = outp.tile([128, J], F32, name="ot", tag=f"ot{i % 2}")
                    for h in range(4):
                        tp = tps.tile([128, 65], F32, name="tp", tag="tp")
                        nc.tensor.transpose(
                            tp[:],
                            ctxs[sc][h][0:65, i * 128 : (i + 1) * 128],
                            ident[0:65, 0:65],
                        )
                        r = rp.tile([128, 1], F32, name="r", tag="r")
                        nc.vector.reciprocal(r[:], tp[:, 64:65])
                        nc.vector.scalar_tensor_tensor(
                            out=ot[:, h * 64 : (h + 1) * 64],
                            in0=tp[:, 0:64],
                            scalar=r[:],
                            in1=bv_t[:, h * 64 : (h + 1) * 64],
                            op0=mybir.AluOpType.mult,
                            op1=mybir.AluOpType.add,
                        )
                    nc.sync.dma_start(
                        out[s0 + i * 128 : s0 + (i + 1) * 128, :], ot[:]
                    )

    nc.compile()
    return nc


def kernel(Q, Wq, bq, Wk, bk, Wv, bv):
    global _cached_nc, last_result
    Q = np.asarray(Q, dtype=np.float32)
    Wq, Wk, Wv = (np.asarray(w, dtype=np.float32) for w in (Wq, Wk, Wv))
    bq, bk, bv = (np.asarray(b, dtype=np.float32) for b in (bq, bk, bv))
    B = Q.shape[0]
    assert Q.shape == (B, S, D) and B * 4 == N_CORES

    if _cached_nc is None:
        _cached_nc = _build()
    nc = _cached_nc

    # host-side shard prep
    qts = [_round_fp32r(Q[b].T) for b in range(B)]
    wqs = [_round_fp32r(Wq[g * J : (g + 1) * J, :].T) for g in range(4)]
    wks = [_round_fp32r(Wk[g * J : (g + 1) * J, :].T) for g in range(4)]
    wvs = [_round_fp32r(Wv[g * J : (g + 1) * J, :].T) for g in range(4)]

    in_maps = []
    for c in range(N_CORES):
        b, g = c // 4, c % 4
        jsl = slice(g * J, (g + 1) * J)
        in_maps.append(
            {
                "qt": qts[b],
                "wq": wqs[g],
                "wk": wks[g],
                "wv": wvs[g],
                "bq": np.ascontiguousarray(bq[jsl]),
                "bk": np.ascontiguousarray(bk[jsl]),
                "bv": np.ascontiguousarray(bv[jsl]),
            }
        )

    last_result = run_bass_kernel_spmd(nc, in_maps, list(range(N_CORES)))

    full = np.empty((B, S, D), dtype=np.float32)
    for c in range(N_CORES):
        b, g = c // 4, c % 4
        full[b, :, g * J : (g + 1) * J] = last_result.results[c]["out"]
    return full


# revision 8
# speedup vs baseline: 1.9724x; 1.2452x over previous
"""Multi-head self-attention Trainium2 kernel (8 NeuronCores, SPMD).

Problem: B=2, S=2048, D=1024, H=16, Dk=64; torch-style Linear projections
(x @ W.T + b), custom softmax: p = exp(scores/8), attn = p / (sum(p) + 1e-8).

Sharding: 32 (batch, head) pairs over 8 cores -> core c handles batch c//4,
heads [4*(c%4), 4*(c%4)+4). Each core projects only its 256 features of
q/k/v; attention is embarrassingly parallel over (b, h).

Per-core kernel (all matmuls in fp32r: fp32 with 11 mantissa bits, ~3x the
fp32 PE throughput, ~1.2e-4 rounding error):
  - inputs (host-prepped): QT = Q[b].T [1024, 2048]; WqT/WkT/WvT [1024, 256]
    (slices of W.T); biases.
  - qT/kT [256, 2048] = (W slice) @ QT + b   (transposed-space projection;
    bias added as a per-partition scalar during the PSUM->SBUF copy)
  - v     [2048, 256] = QT.T @ WvT           (normal layout; bias folded into
    the final normalize: (p@v)/denom + bv, exact because sum_t p*bv = denom*bv)
  - per head pair: scoresT[t, s] two heads packed into the PE array via
    tile_position row groups (0,0)/(64,0); p = exp(scoresT/8) on ScalarE
  - ctxT_ext [65, 512-chunk] = [v_h | 1].T @ p accumulated over 16 t-tiles;
    row 64 = softmax denominator
  - final phase: PE-transpose 128-col blocks -> [128, 65]; DVE reciprocal of
    col 64 and scalar_tensor_tensor: out = ctx * (1/denom) + bv
HAM note: dummy keep-warm matmuls bridge the proj->attention PSUM pool
transition; a >3.4us PE idle there would drop the PE clock to 1.2GHz for the
whole attention phase (observed: 2x slowdown).
Output per core: [2048, 256] fp32 -> host concatenates features per batch.
"""

import sys

sys.path.insert(0, "/opt/trn_rl_repo")

from contextlib import ExitStack

import numpy as np

import concourse.bass as bass
import concourse.tile as tile
from concourse import bacc, mybir
from concourse.bass_utils import run_bass_kernel_spmd
from concourse.masks import make_identity

F32 = mybir.dt.float32
F32R = mybir.dt.float32r

S = 2048  # sequence length
D = 1024  # d_model
J = 256  # features per core (4 heads x 64)
NKT = 8  # k-tiles of the d_model contraction
NSC = 4  # s-chunks of 512
NTT = 16  # t-tiles of 128
N_CORES = 8
N_WARM = 24  # keep-warm dummy matmuls at the proj->attention transition

_cached_nc = None
last_result = None  # BassKernelResults of the most recent run (for test.py)


def _round_fp32r(x: np.ndarray) -> np.ndarray:
    """Round fp32 to fp32r (keep 11 mantissa bits, round to nearest even)."""
    u = np.ascontiguousarray(x, dtype=np.float32).view(np.uint32)
    r = (u.astype(np.uint64) + 0x7FF + ((u >> 12) & 1)) & 0xFFFFF000
    return r.astype(np.uint32).view(np.float32)


def _build():
    nc = bacc.Bacc(None, target_bir_lowering=False)

    qt = nc.dram_tensor("qt", [D, S], F32R, kind="ExternalInput")
    wq = nc.dram_tensor("wq", [D, J], F32R, kind="ExternalInput")
    wk = nc.dram_tensor("wk", [D, J], F32R, kind="ExternalInput")
    wv = nc.dram_tensor("wv", [D, J], F32R, kind="ExternalInput")
    bq = nc.dram_tensor("bq", [J], F32, kind="ExternalInput")
    bk = nc.dram_tensor("bk", [J], F32, kind="ExternalInput")
    bv = nc.dram_tensor("bv", [J], F32, kind="ExternalInput")
    out = nc.dram_tensor("out", [S, J], F32, kind="ExternalOutput")

    with tile.TileContext(nc) as tc, ExitStack() as ctx:
        warm = ctx.enter_context(tc.tile_pool(name="warm", bufs=1, space="PSUM"))
        wts = ctx.enter_context(tc.tile_pool(name="wts", bufs=1))
        qkp = ctx.enter_context(tc.tile_pool(name="qkp", bufs=1))
        vxp = ctx.enter_context(tc.tile_pool(name="vxp", bufs=1))
        bp = ctx.enter_context(tc.tile_pool(name="bp", bufs=1))
        cxp = ctx.enter_context(tc.tile_pool(name="cxp", bufs=1))

        warm_t = warm.tile([128, 512], F32, name="warmt", tag="warm")

        # Weights: 8 k-tiles each of [128, 256]; split across HWDGE (sync) and
        # SWDGE (gpsimd) queues so the first projection matmuls start early
        wq_t, wk_t, wv_t = [], [], []
        for name, dram, lst in (("wq", wq, wq_t), ("wk", wk, wk_t), ("wv", wv, wv_t)):
            for k in range(NKT):
                t = wts.tile([128, J], F32R, name=f"{name}{k}", tag=f"{name}{k}")
                eng = nc.sync if k % 2 == 0 else nc.gpsimd
                eng.dma_start(t[:], dram[k * 128 : (k + 1) * 128, :])
                lst.append(t)

        # Biases: bq/bk as per-partition scalars [128, 2]; bv broadcast [128, 256]
        bq_t = bp.tile([128, 2], F32, name="bqt")
        nc.sync.dma_start(bq_t[:], bq.rearrange("(m p) -> p m", p=128))
        bk_t = bp.tile([128, 2], F32, name="bkt")
        nc.sync.dma_start(bk_t[:], bk.rearrange("(m p) -> p m", p=128))
        bv_t = bp.tile([128, J], F32, name="bvt")
        bvap = bv[:]
        bv_bcast = bass.AP(
            tensor=bvap.tensor, offset=bvap.offset, ap=[[0, 128], [1, J]]
        )
        nc.sync.dma_start(bv_t[:], bv_bcast)

        ident = bp.tile([128, 128], F32, name="ident")
        make_identity(nc, ident[:])
        scratch = bp.tile([128, 1], F32, name="scratch")

        # Persistent projected tensors
        qT = [qkp.tile([128, S], F32R, name=f"qT{m}", tag=f"qT{m}") for m in range(2)]
        kT = [qkp.tile([128, S], F32R, name=f"kT{m}", tag=f"kT{m}") for m in range(2)]
        v_ext = []
        for t in range(NTT):
            vt = vxp.tile([128, 4, 65], F32R, name=f"vx{t}", tag=f"vx{t}")
            nc.gpsimd.memset(vt[:].bitcast(F32), 1.0)  # ones col [:, h, 64] survives
            v_ext.append(vt)
        # ctxT_ext staging for the final phase, per (s-chunk, head)
        ctxs = [
            [cxp.tile([65, 512], F32, name=f"cx{sc}_{h}", tag=f"cx{sc}_{h}")
             for h in range(4)]
            for sc in range(NSC)
        ]

        # ---- Phase 1: projections, pipelined over s-chunks of 512 ----
        with (
            tc.tile_pool(name="qtc", bufs=3) as qtcp,
            tc.tile_pool(name="pps", bufs=1, space="PSUM") as pps,
        ):
            for sc in range(NSC):
                s0 = sc * 512
                qtc = qtcp.tile([128, NKT, 512], F32R, name="qtc", tag="qtc")
                for k in range(NKT):
                    eng = nc.sync if k % 2 == 0 else nc.gpsimd
                    eng.dma_start(
                        qtc[:, k, :], qt[k * 128 : (k + 1) * 128, s0 : s0 + 512]
                    )
                pq = [
                    pps.tile([128, 512], F32, name=f"pq{m}", tag=f"pq{m}")
                    for m in range(2)
                ]
                pk = [
                    pps.tile([128, 512], F32, name=f"pk{m}", tag=f"pk{m}")
                    for m in range(2)
                ]
                # v: two sequential passes of 2 s-subtiles (saves 2 PSUM banks)
                for vpass in range(2):
                    pv = [
                        pps.tile([128, J], F32, name=f"pv{i}", tag=f"pv{i}")
                        for i in range(2)
                    ]
                    for k in range(NKT):
                        st, sp = (k == 0), (k == NKT - 1)
                        if vpass == 0:
                            for m in range(2):
                                nc.tensor.matmul(
                                    pq[m][:],
                                    wq_t[k][:, m * 128 : (m + 1) * 128],
                                    qtc[:, k, :],
                                    start=st,
                                    stop=sp,
                                )
                                nc.tensor.matmul(
                                    pk[m][:],
                                    wk_t[k][:, m * 128 : (m + 1) * 128],
                                    qtc[:, k, :],
                                    start=st,
                                    stop=sp,
                                )
                        for i in range(2):
                            sub = vpass * 2 + i
                            nc.tensor.matmul(
                                pv[i][:],
                                qtc[:, k, sub * 128 : (sub + 1) * 128],
                                wv_t[k][:],
                                start=st,
                                stop=sp,
                            )
                    if vpass == 0:
                        for m in range(2):
                            nc.vector.tensor_scalar_add(
                                qT[m][:, s0 : s0 + 512], pq[m][:], bq_t[:, m : m + 1]
                            )
                            nc.vector.tensor_scalar_add(
                                kT[m][:, s0 : s0 + 512], pk[m][:], bk_t[:, m : m + 1]
                            )
                    for i in range(2):
                        t = sc * 4 + vpass * 2 + i
                        nc.vector.tensor_copy(
                            v_ext[t][:, :, 0:64],
                            pv[i][:].rearrange("p (h d) -> p h d", h=4),
                        )
                if sc == 0:
                    # pre-load the ACT exp table set during projections so the
                    # first attention exp doesn't stall the pipeline ~2.7us
                    nc.scalar.activation(
                        scratch[:], bq_t[:, 0:1],
                        mybir.ActivationFunctionType.Exp, scale=0.0,
                    )

        # keep the PE warm across the PSUM pool transition (released-zone
        # wait is ~4us; >3.4us idle would re-throttle the PE clock)
        for w in range(N_WARM):
            nc.tensor.matmul(
                warm_t[:, 0:J],
                wv_t[0][:, 0:128],
                wv_t[1][:],
                start=True,
                stop=True,
            )

        # ---- Phase 2: attention (scores + exp + AV) ----
        # ACT is the natural bottleneck (1140ns/t vs PE ~920ns/t); a dummy
        # matmul per t-step keeps the PE strictly busier than ACT so the HAM
        # clock gate never sees PE idle and the PE stays at 2.4GHz.
        with (
            tc.tile_pool(name="aps", bufs=1, space="PSUM") as aps,
            tc.tile_pool(name="pTp", bufs=4) as pTp,
        ):
            for sc in range(NSC):
                s0 = sc * 512
                for pair in range(2):
                    hA, hB = 2 * pair, 2 * pair + 1
                    qTt, kTt = qT[pair], kT[pair]
                    ctxA = aps.tile([65, 512], F32, name="ctxA", tag="ctx", bufs=2)
                    ctxB = aps.tile([65, 512], F32, name="ctxB", tag="ctx", bufs=2)
                    pts = {}
                    for t in range(NTT + 1):
                        if t < NTT:
                            tsl = slice(t * 128, (t + 1) * 128)
                            # both heads' scoresT share one 2-bank tile so a
                            # single exp instruction covers them
                            g = aps.tile(
                                [128, 1024], F32, name="g", tag="grp", bufs=2
                            )
                            nc.tensor.matmul(
                                g[:, 0:512],
                                kTt[0:64, tsl],
                                qTt[0:64, s0 : s0 + 512],
                                start=True,
                                stop=True,
                                tile_position=(0, 0),
                            )
                            nc.tensor.matmul(
                                g[:, 512:1024],
                                kTt[64:128, tsl],
                                qTt[64:128, s0 : s0 + 512],
                                start=True,
                                stop=True,
                                tile_position=(64, 0),
                            )
                            pT_ = pTp.tile([128, 1024], F32R, name="pT_", tag="pT")
                            nc.scalar.activation(
                                pT_[:], g[:],
                                mybir.ActivationFunctionType.Exp, scale=0.125,
                            )
                            pts[t] = pT_
                        if t >= 1:
                            pT_ = pts.pop(t - 1)
                            st, sp = (t - 1 == 0), (t - 1 == NTT - 1)
                            nc.tensor.matmul(
                                ctxA[:], v_ext[t - 1][:, hA, :], pT_[:, 0:512],
                                start=st, stop=sp,
                            )
                            nc.tensor.matmul(
                                ctxB[:], v_ext[t - 1][:, hB, :], pT_[:, 512:1024],
                                start=st, stop=sp,
                            )
                        # HAM keep-warm filler
                        nc.tensor.matmul(
                            warm_t[:, 0:J],
                            wv_t[0][:, 0:128],
                            wv_t[1][:],
                            start=True,
                            stop=True,
                        )
                    nc.vector.tensor_copy(ctxs[sc][hA][:], ctxA[:])
                    nc.vector.tensor_copy(ctxs[sc][hB][:], ctxB[:])

        # ---- Phase 3: transpose + normalize + bias + store ----
        with (
            tc.tile_pool(name="tps", bufs=4, space="PSUM") as tps,
            tc.tile_pool(name="outp", bufs=2) as outp,
            tc.tile_pool(name="rp", bufs=8) as rp,
        ):
            for sc in range(NSC):
                s0 = sc * 512
                for i in range(4):
                    ot = outp.tile([128, J], F32, name="ot", tag=f"ot{i % 2}")
                    for h in range(4):
                        tp = tps.tile([128, 65], F32, name="tp", tag="tp")
                        nc.tensor.transpose(
                            tp[:],
                            ctxs[sc][h][0:65, i * 128 : (i + 1) * 128],
                            ident[0:65, 0:65],
                        )
                        r = rp.tile([128, 1], F32, name="r", tag="r")
                        nc.vector.reciprocal(r[:], tp[:, 64:65])
                        nc.vector.scalar_tensor_tensor(
                            out=ot[:, h * 64 : (h + 1) * 64],
                            in0=tp[:, 0:64],
                            scalar=r[:],
                            in1=bv_t[:, h * 64 : (h + 1) * 64],
                            op0=mybir.AluOpType.mult,
                            op1=mybir.AluOpType.add,
                        )
                    nc.sync.dma_start(
                        out[s0 + i * 128 : s0 + (i + 1) * 128, :], ot[:]
                    )

    nc.compile()
    return nc


def kernel(Q, Wq, bq, Wk, bk, Wv, bv):
    global _cached_nc, last_result
    Q = np.asarray(Q, dtype=np.float32)
    Wq, Wk, Wv = (np.asarray(w, dtype=np.float32) for w in (Wq, Wk, Wv))
    bq, bk, bv = (np.asarray(b, dtype=np.float32) for b in (bq, bk, bv))
    B = Q.shape[0]
    assert Q.shape == (B, S, D) and B * 4 == N_CORES

    if _cached_nc is None:
        _cached_nc = _build()
    nc = _cached_nc

    # host-side shard prep
    qts = [_round_fp32r(Q[b].T) for b in range(B)]
    wqs = [_round_fp32r(Wq[g * J : (g + 1) * J, :].T) for g in range(4)]
    wks = [_round_fp32r(Wk[g * J : (g + 1) * J, :].T) for g in range(4)]
    wvs = [_round_fp32r(Wv[g * J : (g + 1) * J, :].T) for g in range(4)]

    in_maps = []
    for c in range(N_CORES):
        b, g = c // 4, c % 4
        jsl = slice(g * J, (g + 1) * J)
        in_maps.append(
            {
                "qt": qts[b],
                "wq": wqs[g],
                "wk": wks[g],
                "wv": wvs[g],
                "bq": np.ascontiguousarray(bq[jsl]),
                "bk": np.ascontiguousarray(bk[jsl]),
                "bv": np.ascontiguousarray(bv[jsl]),
            }
        )

    last_result = run_bass_kernel_spmd(nc, in_maps, list(range(N_CORES)))

    full = np.empty((B, S, D), dtype=np.float32)
    for c in range(N_CORES):
        b, g = c // 4, c % 4
        full[b, :, g * J : (g + 1) * J] = last_result.results[c]["out"]
    return full


# revision 9
# speedup vs baseline: 2.1106x; 1.0701x over previous
"""Multi-head self-attention Trainium2 kernel (8 NeuronCores, SPMD).

Problem: B=2, S=2048, D=1024, H=16, Dk=64; torch-style Linear projections
(x @ W.T + b), custom softmax: p = exp(scores/8), attn = p / (sum(p) + 1e-8).

Sharding: 32 (batch, head) pairs over 8 cores -> core c handles batch c//4,
heads [4*(c%4), 4*(c%4)+4). Each core projects only its 256 features of
q/k/v; attention is embarrassingly parallel over (b, h).

Per-core kernel (all matmuls in fp32r: fp32 with 11 mantissa bits, ~3x the
fp32 PE throughput, ~1.2e-4 rounding error):
  - inputs (host-prepped): QT = Q[b].T [1024, 2048]; WqT/WkT/WvT [1024, 256]
    (slices of W.T); biases.
  - qT/kT [256, 2048] = (W slice) @ QT + b   (transposed-space projection;
    bias added as a per-partition scalar during the PSUM->SBUF copy)
  - v     [2048, 256] = QT.T @ WvT           (normal layout; bias folded into
    the final normalize: (p@v)/denom + bv, exact because sum_t p*bv = denom*bv)
  - per head pair: scoresT[t, s] for both heads packed into one PE pass via
    tile_position row groups (0,0)/(64,0), written into one 2-bank PSUM tile
    so a single exp instruction [128,1024] covers both heads (ScalarE is the
    bottleneck engine; its fixed ~0.5us/instruction overhead is halved)
  - ctxT_ext [65, 512-chunk] = [v_h | 1].T @ p accumulated over 16 t-tiles;
    row 64 = softmax denominator
  - finalize: PE-transpose 128-col blocks -> [128, 65]; DVE reciprocal of
    col 64 and scalar_tensor_tensor: out = ctx * (1/denom) + bv

Scheduling: the attention phase is ACT(exp)-bound (~1.3us per t-step); the
PE's spare capacity there is filled with useful work -- the pair-1
projections and the transpose/normalize pipeline -- one or two units per
t-step. This both hides that work entirely and keeps the PE busy enough
that the HAM clock gate never re-throttles it to 1.2GHz (a >3.4us PE idle
anywhere would double every subsequent matmul's duration; observed).

Output per core: [2048, 256] fp32 -> host concatenates features per batch.
"""

import sys

sys.path.insert(0, "/opt/trn_rl_repo")

from collections import deque
from contextlib import ExitStack

import numpy as np

import concourse.bass as bass
import concourse.tile as tile
from concourse import bacc, mybir
from concourse.bass_utils import run_bass_kernel_spmd
from concourse.masks import make_identity

F32 = mybir.dt.float32
F32R = mybir.dt.float32r

S = 2048  # sequence length
D = 1024  # d_model
J = 256  # features per core (4 heads x 64)
NKT = 8  # k-tiles of the d_model contraction
NSC = 4  # s-chunks of 512
NTT = 16  # t-tiles of 128
N_CORES = 8

_cached_nc = None
last_result = None  # BassKernelResults of the most recent run (for test.py)


def _round_fp32r(x: np.ndarray) -> np.ndarray:
    """Round fp32 to fp32r (keep 11 mantissa bits, round to nearest even)."""
    u = np.ascontiguousarray(x, dtype=np.float32).view(np.uint32)
    r = (u.astype(np.uint64) + 0x7FF + ((u >> 12) & 1)) & 0xFFFFF000
    return r.astype(np.uint32).view(np.float32)


def _build():
    nc = bacc.Bacc(None, target_bir_lowering=False)

    qt = nc.dram_tensor("qt", [D, S], F32R, kind="ExternalInput")
    wq = nc.dram_tensor("wq", [D, J], F32R, kind="ExternalInput")
    wk = nc.dram_tensor("wk", [D, J], F32R, kind="ExternalInput")
    wv = nc.dram_tensor("wv", [D, J], F32R, kind="ExternalInput")
    bq = nc.dram_tensor("bq", [J], F32, kind="ExternalInput")
    bk = nc.dram_tensor("bk", [J], F32, kind="ExternalInput")
    bv = nc.dram_tensor("bv", [J], F32, kind="ExternalInput")
    out = nc.dram_tensor("out", [S, J], F32, kind="ExternalOutput")

    with tile.TileContext(nc) as tc, ExitStack() as ctx:
        wts = ctx.enter_context(tc.tile_pool(name="wts", bufs=1))
        qkp = ctx.enter_context(tc.tile_pool(name="qkp", bufs=1))
        vxp = ctx.enter_context(tc.tile_pool(name="vxp", bufs=1))
        bp = ctx.enter_context(tc.tile_pool(name="bp", bufs=1))
        cxp = ctx.enter_context(tc.tile_pool(name="cxp", bufs=8))
        pTp = ctx.enter_context(tc.tile_pool(name="pTp", bufs=4))
        outp = ctx.enter_context(tc.tile_pool(name="outp", bufs=1))
        rp = ctx.enter_context(tc.tile_pool(name="rp", bufs=8))
        qtcp = ctx.enter_context(tc.tile_pool(name="qtc", bufs=3))

        # Weights: 8 k-tiles each of [128, 256]; split across HWDGE (sync) and
        # SWDGE (gpsimd) queues so the first projection matmuls start early
        wq_t, wk_t, wv_t = [], [], []
        for name, dram, lst in (("wq", wq, wq_t), ("wk", wk, wk_t), ("wv", wv, wv_t)):
            for k in range(NKT):
                t = wts.tile([128, J], F32R, name=f"{name}{k}", tag=f"{name}{k}")
                eng = nc.sync if (k + len(lst)) % 2 == 0 else nc.gpsimd
                eng.dma_start(t[:], dram[k * 128 : (k + 1) * 128, :])
                lst.append(t)

        # Biases: bq/bk as per-partition scalars [128, 2]; bv broadcast [128, 256]
        bq_t = bp.tile([128, 2], F32, name="bqt")
        nc.sync.dma_start(bq_t[:], bq.rearrange("(m p) -> p m", p=128))
        bk_t = bp.tile([128, 2], F32, name="bkt")
        nc.sync.dma_start(bk_t[:], bk.rearrange("(m p) -> p m", p=128))
        bv_t = bp.tile([128, J], F32, name="bvt")
        bvap = bv[:]
        bv_bcast = bass.AP(
            tensor=bvap.tensor, offset=bvap.offset, ap=[[0, 128], [1, J]]
        )
        nc.sync.dma_start(bv_t[:], bv_bcast)

        ident = bp.tile([128, 128], F32, name="ident")
        make_identity(nc, ident[:])
        scratch = bp.tile([128, 1], F32, name="scratch")

        # Persistent projected tensors
        qT = [qkp.tile([128, S], F32R, name=f"qT{m}", tag=f"qT{m}") for m in range(2)]
        kT = [qkp.tile([128, S], F32R, name=f"kT{m}", tag=f"kT{m}") for m in range(2)]
        v_ext = []
        for t in range(NTT):
            vt = vxp.tile([128, 4, 65], F32R, name=f"vx{t}", tag=f"vx{t}")
            nc.gpsimd.memset(vt[:].bitcast(F32), 1.0)  # ones col [:, h, 64] survives
            v_ext.append(vt)
        # out accumulation tiles, one per 128-row block of the output
        out_tiles = [
            outp.tile([128, J], F32, name=f"ot{b}", tag=f"ot{b}") for b in range(16)
        ]

        def dma_qtc(tile_, sc):
            s0 = sc * 512
            for k in range(NKT):
                eng = nc.sync if k % 2 == 0 else nc.gpsimd
                eng.dma_start(
                    tile_[:, k, :], qt[k * 128 : (k + 1) * 128, s0 : s0 + 512]
                )

        # ---- Phase 1: pair-0 projections (qT[0], kT[0]) + all of v ----
        with tc.tile_pool(name="pps", bufs=1, space="PSUM") as pps:
            for sc in range(NSC):
                s0 = sc * 512
                qtc = qtcp.tile([128, NKT, 512], F32R, name="qtc", tag="qtc")
                dma_qtc(qtc, sc)
                pq = pps.tile([128, 512], F32, name="pq", tag="pq")
                pk = pps.tile([128, 512], F32, name="pk", tag="pk")
                pv = [
                    pps.tile([128, J], F32, name=f"pv{i}", tag=f"pv{i}")
                    for i in range(4)
                ]
                for k in range(NKT):
                    st, sp = (k == 0), (k == NKT - 1)
                    nc.tensor.matmul(
                        pq[:], wq_t[k][:, 0:128], qtc[:, k, :], start=st, stop=sp
                    )
                    nc.tensor.matmul(
                        pk[:], wk_t[k][:, 0:128], qtc[:, k, :], start=st, stop=sp
                    )
                    for i in range(4):
                        nc.tensor.matmul(
                            pv[i][:],
                            qtc[:, k, i * 128 : (i + 1) * 128],
                            wv_t[k][:],
                            start=st,
                            stop=sp,
                        )
                nc.vector.tensor_scalar_add(
                    qT[0][:, s0 : s0 + 512], pq[:], bq_t[:, 0:1]
                )
                nc.vector.tensor_scalar_add(
                    kT[0][:, s0 : s0 + 512], pk[:], bk_t[:, 0:1]
                )
                for i in range(4):
                    nc.vector.tensor_copy(
                        v_ext[sc * 4 + i][:, :, 0:64],
                        pv[i][:].rearrange("p (h d) -> p h d", h=4),
                    )
                if sc == 0:
                    # pre-load the ACT exp table set during projections so the
                    # first attention exp doesn't stall the pipeline ~2.7us
                    nc.scalar.activation(
                        scratch[:], bq_t[:, 0:1],
                        mybir.ActivationFunctionType.Exp, scale=0.0,
                    )

        # ---- Phase 2: attention, with pair-1 projections and the
        #      transpose/normalize pipeline as PE filler work ----
        with (
            tc.tile_pool(name="aps", bufs=1, space="PSUM") as aps,
            tc.tile_pool(name="p1b", bufs=1, space="PSUM") as p1b,
        ):
            # --- filler: pair-1 projection work units ---
            p1_state = {}

            def u_dma(c):
                def f():
                    qtc2 = qtcp.tile([128, NKT, 512], F32R, name="qtc2", tag="qtc")
                    dma_qtc(qtc2, c)
                    px0 = p1b.tile([128, 512], F32, name="px0", tag="x0")
                    px1 = p1b.tile([128, 512], F32, name="px1", tag="x1")
                    p1_state[c] = (qtc2, px0, px1)
                return f

            def u_k(c, k):
                def f():
                    qtc2, px0, px1 = p1_state[c]
                    st, sp = (k == 0), (k == NKT - 1)
                    nc.tensor.matmul(
                        px0[:], wq_t[k][:, 128:256], qtc2[:, k, :], start=st, stop=sp
                    )
                    nc.tensor.matmul(
                        px1[:], wk_t[k][:, 128:256], qtc2[:, k, :], start=st, stop=sp
                    )
                return f

            def u_copy(c):
                def f():
                    _, px0, px1 = p1_state.pop(c)
                    s0 = c * 512
                    nc.vector.tensor_scalar_add(
                        qT[1][:, s0 : s0 + 512], px0[:], bq_t[:, 1:2]
                    )
                    nc.vector.tensor_scalar_add(
                        kT[1][:, s0 : s0 + 512], px1[:], bk_t[:, 1:2]
                    )
                return f

            work = deque()
            for c in range(NSC):
                work.append(u_dma(c))
                for k in range(NKT):
                    work.append(u_k(c, k))
                work.append(u_copy(c))

            # --- filler: transpose/normalize pieces ---
            pieces = deque()
            done_cnt = {}
            piece_idx = [0]

            def piece(cs_tile, sc, h, i):
                def f():
                    tagidx = piece_idx[0] % 2
                    piece_idx[0] += 1
                    tp = p1b.tile(
                        [128, 65], F32, name="tp", tag=f"x{tagidx}"
                    )
                    nc.tensor.transpose(
                        tp[:],
                        cs_tile[0:65, i * 128 : (i + 1) * 128],
                        ident[0:65, 0:65],
                    )
                    r = rp.tile([128, 1], F32, name="r", tag="r")
                    nc.vector.reciprocal(r[:], tp[:, 64:65])
                    blk = sc * 4 + i
                    nc.vector.scalar_tensor_tensor(
                        out=out_tiles[blk][:, h * 64 : (h + 1) * 64],
                        in0=tp[:, 0:64],
                        scalar=r[:],
                        in1=bv_t[:, h * 64 : (h + 1) * 64],
                        op0=mybir.AluOpType.mult,
                        op1=mybir.AluOpType.add,
                    )
                    done_cnt[blk] = done_cnt.get(blk, 0) + 1
                    if done_cnt[blk] == 4:
                        nc.sync.dma_start(
                            out[blk * 128 : (blk + 1) * 128, :], out_tiles[blk][:]
                        )
                return f

            def fill_slot():
                # pair-1 projections first (2 units/slot: they gate the
                # pair-1 attention blocks), then transpose pieces, which
                # reuse the x0/x1 PSUM banks after the projections retire
                if work:
                    work.popleft()()
                    if work:
                        work.popleft()()
                elif pieces:
                    pieces.popleft()()
                    if len(pieces) > 12 and pieces:
                        pieces.popleft()()

            # burst: pair-1 chunk 0 bridges the PSUM pool-transition wait so
            # the PE never idles across the phase boundary (HAM)
            for _ in range(10):
                if work:
                    work.popleft()()

            for pair in range(2):
                for sc in range(NSC):
                    s0 = sc * 512
                    hA, hB = 2 * pair, 2 * pair + 1
                    qTt, kTt = qT[pair], kT[pair]
                    ctxA = aps.tile([65, 512], F32, name="ctxA", tag="ctx", bufs=2)
                    ctxB = aps.tile([65, 512], F32, name="ctxB", tag="ctx", bufs=2)
                    pts = {}
                    for t in range(NTT + 1):
                        if t < NTT:
                            tsl = slice(t * 128, (t + 1) * 128)
                            # both heads' scoresT share one 2-bank tile so a
                            # single exp instruction covers them
                            g = aps.tile(
                                [128, 1024], F32, name="g", tag="grp", bufs=2
                            )
                            nc.tensor.matmul(
                                g[:, 0:512],
                                kTt[0:64, tsl],
                                qTt[0:64, s0 : s0 + 512],
                                start=True,
                                stop=True,
                                tile_position=(0, 0),
                            )
                            nc.tensor.matmul(
                                g[:, 512:1024],
                                kTt[64:128, tsl],
                                qTt[64:128, s0 : s0 + 512],
                                start=True,
                                stop=True,
                                tile_position=(64, 0),
                            )
                            pT_ = pTp.tile([128, 1024], F32R, name="pT_", tag="pT")
                            nc.scalar.activation(
                                pT_[:], g[:],
                                mybir.ActivationFunctionType.Exp, scale=0.125,
                            )
                            pts[t] = pT_
                        if t >= 1:
                            pT_ = pts.pop(t - 1)
                            st, sp = (t - 1 == 0), (t - 1 == NTT - 1)
                            nc.tensor.matmul(
                                ctxA[:], v_ext[t - 1][:, hA, :], pT_[:, 0:512],
                                start=st, stop=sp,
                            )
                            nc.tensor.matmul(
                                ctxB[:], v_ext[t - 1][:, hB, :], pT_[:, 512:1024],
                                start=st, stop=sp,
                            )
                        fill_slot()
                    csA = cxp.tile([65, 512], F32, name="csA", tag="cs")
                    nc.vector.tensor_copy(csA[:], ctxA[:])
                    csB = cxp.tile([65, 512], F32, name="csB", tag="cs")
                    nc.vector.tensor_copy(csB[:], ctxB[:])
                    for i in range(4):
                        pieces.append(piece(csA, sc, hA, i))
                        pieces.append(piece(csB, sc, hB, i))

            # drain remaining filler work
            while work:
                work.popleft()()
            while pieces:
                pieces.popleft()()

    nc.compile()
    return nc


def kernel(Q, Wq, bq, Wk, bk, Wv, bv):
    global _cached_nc, last_result
    Q = np.asarray(Q, dtype=np.float32)
    Wq, Wk, Wv = (np.asarray(w, dtype=np.float32) for w in (Wq, Wk, Wv))
    bq, bk, bv = (np.asarray(b, dtype=np.float32) for b in (bq, bk, bv))
    B = Q.shape[0]
    assert Q.shape == (B, S, D) and B * 4 == N_CORES

    if _cached_nc is None:
        _cached_nc = _build()
    nc = _cached_nc

    # host-side shard prep
    qts = [_round_fp32r(Q[b].T) for b in range(B)]
    wqs = [_round_fp32r(Wq[g * J : (g + 1) * J, :].T) for g in range(4)]
    wks = [_round_fp32r(Wk[g * J : (g + 1) * J, :].T) for g in range(4)]
    wvs = [_round_fp32r(Wv[g * J : (g + 1) * J, :].T) for g in range(4)]

    in_maps = []
    for c in range(N_CORES):
        b, g = c // 4, c % 4
        jsl = slice(g * J, (g + 1) * J)
        in_maps.append(
            {
                "qt": qts[b],
                "wq": wqs[g],
                "wk": wks[g],
                "wv": wvs[g],
                "bq": np.ascontiguousarray(bq[jsl]),
                "bk": np.ascontiguousarray(bk[jsl]),
                "bv": np.ascontiguousarray(bv[jsl]),
            }
        )

    last_result = run_bass_kernel_spmd(nc, in_maps, list(range(N_CORES)))

    full = np.empty((B, S, D), dtype=np.float32)
    for c in range(N_CORES):
        b, g = c // 4, c % 4
        full[b, :, g * J : (g + 1) * J] = last_result.results[c]["out"]
    return full


# revision 12
# speedup vs baseline: 2.2425x; 1.0625x over previous
"""Multi-head self-attention Trainium2 kernel (8 NeuronCores, SPMD).

Problem: B=2, S=2048, D=1024, H=16, Dk=64; torch-style Linear projections
(x @ W.T + b), custom softmax: p = exp(scores/8), attn = p / (sum(p) + 1e-8).

Sharding: 32 (batch, head) pairs over 8 cores -> core c handles batch c//4,
heads [4*(c%4), 4*(c%4)+4). Each core projects only its 256 features of
q/k/v; attention is embarrassingly parallel over (b, h).

Per-core kernel (all matmuls in fp32r: fp32 with 11 mantissa bits, ~3x the
fp32 PE throughput, ~1.2e-4 rounding error):
  - inputs (host-prepped): QT = Q[b].T [1024, 2048]; WqT/WkT/WvT [1024, 256]
    (slices of W.T); biases.
  - qT/kT [256, 2048] = (W slice) @ QT + b   (transposed-space projection;
    bias added as a per-partition scalar during the PSUM->SBUF copy)
  - v     [2048, 256] = QT.T @ WvT           (normal layout; bias folded into
    the final normalize: (p@v)/denom + bv, exact because sum_t p*bv = denom*bv)
  - per head pair: scoresT[t, s] for both heads packed into one PE pass via
    tile_position row groups (0,0)/(64,0), written into one 2-bank PSUM tile
    so a single exp instruction [128,1024] covers both heads (ScalarE is the
    bottleneck engine; its fixed ~0.5us/instruction overhead is halved)
  - ctxT_ext [65, 512-chunk] = [v_h | 1].T @ p accumulated over 16 t-tiles;
    row 64 = softmax denominator
  - finalize: PE-transpose 128-col blocks -> [128, 65]; DVE reciprocal of
    col 64 and scalar_tensor_tensor: out = ctx * (1/denom) + bv

Scheduling: the attention phase is ACT(exp)-bound (~1.3us per t-step); the
PE's spare capacity there is filled with useful work -- the pair-1
projections and the transpose/normalize pipeline -- one or two units per
t-step. This both hides that work entirely and keeps the PE busy enough
that the HAM clock gate never re-throttles it to 1.2GHz (a >3.4us PE idle
anywhere would double every subsequent matmul's duration; observed).

Output per core: [2048, 256] fp32 -> host concatenates features per batch.
"""

import sys

sys.path.insert(0, "/opt/trn_rl_repo")

from collections import deque
from contextlib import ExitStack

import numpy as np

import concourse.bass as bass
import concourse.tile as tile
from concourse import bacc, mybir
from concourse.bass_utils import run_bass_kernel_spmd
from concourse.masks import make_identity

F32 = mybir.dt.float32
F32R = mybir.dt.float32r

S = 2048  # sequence length
D = 1024  # d_model
J = 256  # features per core (4 heads x 64)
NKT = 8  # k-tiles of the d_model contraction
NSC = 4  # s-chunks of 512
NTT = 16  # t-tiles of 128
N_CORES = 8

_cached_nc = None
last_result = None  # BassKernelResults of the most recent run (for test.py)


def _round_fp32r(x: np.ndarray) -> np.ndarray:
    """Round fp32 to fp32r (keep 11 mantissa bits, round to nearest even)."""
    u = np.ascontiguousarray(x, dtype=np.float32).view(np.uint32)
    r = (u.astype(np.uint64) + 0x7FF + ((u >> 12) & 1)) & 0xFFFFF000
    return r.astype(np.uint32).view(np.float32)


def _build():
    nc = bacc.Bacc(None, target_bir_lowering=False)

    qt = nc.dram_tensor("qt", [D, S], F32R, kind="ExternalInput")
    wq = nc.dram_tensor("wq", [D, J], F32R, kind="ExternalInput")
    wk = nc.dram_tensor("wk", [D, J], F32R, kind="ExternalInput")
    wv = nc.dram_tensor("wv", [D, J], F32R, kind="ExternalInput")
    bq = nc.dram_tensor("bq", [J], F32, kind="ExternalInput")
    bk = nc.dram_tensor("bk", [J], F32, kind="ExternalInput")
    bv = nc.dram_tensor("bv", [J], F32, kind="ExternalInput")
    out = nc.dram_tensor("out", [S, J], F32, kind="ExternalOutput")

    with tile.TileContext(nc) as tc, ExitStack() as ctx:
        wts = ctx.enter_context(tc.tile_pool(name="wts", bufs=1))
        qkp = ctx.enter_context(tc.tile_pool(name="qkp", bufs=1))
        vxp = ctx.enter_context(tc.tile_pool(name="vxp", bufs=1))
        bp = ctx.enter_context(tc.tile_pool(name="bp", bufs=1))
        cxp = ctx.enter_context(tc.tile_pool(name="cxp", bufs=8))
        pTp = ctx.enter_context(tc.tile_pool(name="pTp", bufs=4))
        outp = ctx.enter_context(tc.tile_pool(name="outp", bufs=1))
        rp = ctx.enter_context(tc.tile_pool(name="rp", bufs=8))
        qtcp = ctx.enter_context(tc.tile_pool(name="qtc", bufs=3))

        # Weights: 8 k-tiles each of [128, 256], k-major and split across the
        # HWDGE (sync) / SWDGE (gpsimd) queues, interleaved with the first
        # s-chunk of QT below so the k=0 projection matmuls start early
        wq_t = [
            wts.tile([128, J], F32R, name=f"wq{k}", tag=f"wq{k}") for k in range(NKT)
        ]
        wk_t = [
            wts.tile([128, J], F32R, name=f"wk{k}", tag=f"wk{k}") for k in range(NKT)
        ]
        wv_t = [
            wts.tile([128, J], F32R, name=f"wv{k}", tag=f"wv{k}") for k in range(NKT)
        ]
        qtc0 = qtcp.tile([128, NKT, 512], F32R, name="qtc0", tag="qtc")
        for k in range(NKT):
            ksl = slice(k * 128, (k + 1) * 128)
            nc.sync.dma_start(qtc0[:, k, :], qt[ksl, 0:512])
            nc.gpsimd.dma_start(wq_t[k][:], wq[ksl, :])
            nc.sync.dma_start(wk_t[k][:], wk[ksl, :])
            nc.gpsimd.dma_start(wv_t[k][:], wv[ksl, :])

        # Biases: bq/bk as per-partition scalars [128, 2]; bv broadcast [128, 256]
        bq_t = bp.tile([128, 2], F32, name="bqt")
        nc.sync.dma_start(bq_t[:], bq.rearrange("(m p) -> p m", p=128))
        bk_t = bp.tile([128, 2], F32, name="bkt")
        nc.sync.dma_start(bk_t[:], bk.rearrange("(m p) -> p m", p=128))
        bv_t = bp.tile([128, J], F32, name="bvt")
        bvap = bv[:]
        bv_bcast = bass.AP(
            tensor=bvap.tensor, offset=bvap.offset, ap=[[0, 128], [1, J]]
        )
        nc.sync.dma_start(bv_t[:], bv_bcast)

        ident = bp.tile([128, 128], F32, name="ident")
        make_identity(nc, ident[:])
        scratch = bp.tile([128, 1], F32, name="scratch")

        # Persistent projected tensors
        qT = [qkp.tile([128, S], F32R, name=f"qT{m}", tag=f"qT{m}") for m in range(2)]
        kT = [qkp.tile([128, S], F32R, name=f"kT{m}", tag=f"kT{m}") for m in range(2)]
        v_ext = []
        for t in range(NTT):
            vt = vxp.tile([128, 4, 65], F32R, name=f"vx{t}", tag=f"vx{t}")
            nc.gpsimd.memset(vt[:].bitcast(F32), 1.0)  # ones col [:, h, 64] survives
            v_ext.append(vt)
        # out accumulation tiles, one per 128-row block of the output
        out_tiles = [
            outp.tile([128, J], F32, name=f"ot{b}", tag=f"ot{b}") for b in range(16)
        ]

        def dma_qtc(tile_, sc):
            s0 = sc * 512
            for k in range(NKT):
                eng = nc.sync if k % 2 == 0 else nc.gpsimd
                eng.dma_start(
                    tile_[:, k, :], qt[k * 128 : (k + 1) * 128, s0 : s0 + 512]
                )

        # ---- Phase 1: pair-0 projections (qT[0], kT[0]) + all of v ----
        with tc.tile_pool(name="pps", bufs=1, space="PSUM") as pps:
            for sc in range(NSC):
                s0 = sc * 512
                if sc == 0:
                    qtc = qtc0
                else:
                    qtc = qtcp.tile([128, NKT, 512], F32R, name="qtc", tag="qtc")
                    dma_qtc(qtc, sc)
                pq = pps.tile([128, 512], F32, name="pq", tag="pq")
                pk = pps.tile([128, 512], F32, name="pk", tag="pk")
                pv = [
                    pps.tile([128, J], F32, name=f"pv{i}", tag=f"pv{i}")
                    for i in range(4)
                ]
                for k in range(NKT):
                    st, sp = (k == 0), (k == NKT - 1)
                    nc.tensor.matmul(
                        pq[:], wq_t[k][:, 0:128], qtc[:, k, :], start=st, stop=sp
                    )
                    nc.tensor.matmul(
                        pk[:], wk_t[k][:, 0:128], qtc[:, k, :], start=st, stop=sp
                    )
                    for i in range(4):
                        nc.tensor.matmul(
                            pv[i][:],
                            qtc[:, k, i * 128 : (i + 1) * 128],
                            wv_t[k][:],
                            start=st,
                            stop=sp,
                        )
                nc.vector.tensor_scalar_add(
                    qT[0][:, s0 : s0 + 512], pq[:], bq_t[:, 0:1]
                )
                nc.vector.tensor_scalar_add(
                    kT[0][:, s0 : s0 + 512], pk[:], bk_t[:, 0:1]
                )
                for i in range(4):
                    nc.vector.tensor_copy(
                        v_ext[sc * 4 + i][:, :, 0:64],
                        pv[i][:].rearrange("p (h d) -> p h d", h=4),
                    )
                if sc == 0:
                    # pre-load the ACT exp table set during projections so the
                    # first attention exp doesn't stall the pipeline ~2.7us
                    nc.scalar.activation(
                        scratch[:], bq_t[:, 0:1],
                        mybir.ActivationFunctionType.Exp, scale=0.0,
                    )

        # ---- Phase 2: attention, with pair-1 projections and the
        #      transpose/normalize pipeline as PE filler work ----
        with (
            tc.tile_pool(name="aps", bufs=1, space="PSUM") as aps,
            tc.tile_pool(name="p1b", bufs=1, space="PSUM") as p1b,
        ):
            # --- filler: pair-1 projection work units ---
            p1_state = {}

            def u_dma(c):
                def f():
                    qtc2 = qtcp.tile([128, NKT, 512], F32R, name="qtc2", tag="qtc")
                    dma_qtc(qtc2, c)
                    px0 = p1b.tile([128, 512], F32, name="px0", tag="x0")
                    px1 = p1b.tile([128, 512], F32, name="px1", tag="x1")
                    p1_state[c] = (qtc2, px0, px1)
                return f

            def u_k(c, k):
                def f():
                    qtc2, px0, px1 = p1_state[c]
                    st, sp = (k == 0), (k == NKT - 1)
                    nc.tensor.matmul(
                        px0[:], wq_t[k][:, 128:256], qtc2[:, k, :], start=st, stop=sp
                    )
                    nc.tensor.matmul(
                        px1[:], wk_t[k][:, 128:256], qtc2[:, k, :], start=st, stop=sp
                    )
                return f

            def u_copy(c):
                def f():
                    _, px0, px1 = p1_state.pop(c)
                    s0 = c * 512
                    nc.vector.tensor_scalar_add(
                        qT[1][:, s0 : s0 + 512], px0[:], bq_t[:, 1:2]
                    )
                    nc.vector.tensor_scalar_add(
                        kT[1][:, s0 : s0 + 512], px1[:], bk_t[:, 1:2]
                    )
                return f

            work = deque()
            for c in range(NSC):
                work.append(u_dma(c))
                for k in range(NKT):
                    work.append(u_k(c, k))
                work.append(u_copy(c))

            # --- filler: transpose/normalize pieces ---
            pieces = deque()
            done_cnt = {}
            piece_idx = [0]

            def piece(cs_tile, sc, h, i):
                def f():
                    tagidx = piece_idx[0] % 2
                    piece_idx[0] += 1
                    tp = p1b.tile(
                        [128, 65], F32, name="tp", tag=f"x{tagidx}"
                    )
                    nc.tensor.transpose(
                        tp[:],
                        cs_tile[0:65, i * 128 : (i + 1) * 128],
                        ident[0:65, 0:65],
                    )
                    r = rp.tile([128, 1], F32, name="r", tag="r")
                    nc.vector.reciprocal(r[:], tp[:, 64:65])
                    blk = sc * 4 + i
                    nc.vector.scalar_tensor_tensor(
                        out=out_tiles[blk][:, h * 64 : (h + 1) * 64],
                        in0=tp[:, 0:64],
                        scalar=r[:],
                        in1=bv_t[:, h * 64 : (h + 1) * 64],
                        op0=mybir.AluOpType.mult,
                        op1=mybir.AluOpType.add,
                    )
                    done_cnt[blk] = done_cnt.get(blk, 0) + 1
                    if done_cnt[blk] == 4:
                        nc.sync.dma_start(
                            out[blk * 128 : (blk + 1) * 128, :], out_tiles[blk][:]
                        )
                return f

            def fill_slot():
                # pair-1 projections first (they gate the pair-1 attention
                # blocks), then transpose pieces, which reuse the x0/x1 PSUM
                # banks after the projections retire
                if work:
                    work.popleft()()
                elif pieces:
                    pieces.popleft()()
                    if len(pieces) > 12 and pieces:
                        pieces.popleft()()

            # burst: pair-1 chunk 0 bridges the PSUM pool-transition wait so
            # the PE never idles across the phase boundary (HAM)
            for _ in range(10):
                if work:
                    work.popleft()()

            for pair in range(2):
                for sc in range(NSC):
                    s0 = sc * 512
                    hA, hB = 2 * pair, 2 * pair + 1
                    qTt, kTt = qT[pair], kT[pair]
                    ctxA = aps.tile([65, 512], F32, name="ctxA", tag="ctx", bufs=2)
                    ctxB = aps.tile([65, 512], F32, name="ctxB", tag="ctx", bufs=2)
                    pts = {}
                    for t in range(NTT + 1):
                        if t < NTT:
                            tsl = slice(t * 128, (t + 1) * 128)
                            # both heads' scoresT share one 2-bank tile so a
                            # single exp instruction covers them
                            g = aps.tile(
                                [128, 1024], F32, name="g", tag="grp", bufs=2
                            )
                            nc.tensor.matmul(
                                g[:, 0:512],
                                kTt[0:64, tsl],
                                qTt[0:64, s0 : s0 + 512],
                                start=True,
                                stop=True,
                                tile_position=(0, 0),
                            )
                            nc.tensor.matmul(
                                g[:, 512:1024],
                                kTt[64:128, tsl],
                                qTt[64:128, s0 : s0 + 512],
                                start=True,
                                stop=True,
                                tile_position=(64, 0),
                            )
                            pT_ = pTp.tile([128, 1024], F32R, name="pT_", tag="pT")
                            nc.scalar.activation(
                                pT_[:], g[:],
                                mybir.ActivationFunctionType.Exp, scale=0.125,
                            )
                            pts[t] = pT_
                        if t >= 1:
                            pT_ = pts.pop(t - 1)
                            st, sp = (t - 1 == 0), (t - 1 == NTT - 1)
                            nc.tensor.matmul(
                                ctxA[:], v_ext[t - 1][:, hA, :], pT_[:, 0:512],
                                start=st, stop=sp,
                            )
                            nc.tensor.matmul(
                                ctxB[:], v_ext[t - 1][:, hB, :], pT_[:, 512:1024],
                                start=st, stop=sp,
                            )
                        fill_slot()
                    csA = cxp.tile([65, 512], F32, name="csA", tag="cs")
                    nc.vector.tensor_copy(csA[:], ctxA[:])
                    csB = cxp.tile([65, 512], F32, name="csB", tag="cs")
                    nc.vector.tensor_copy(csB[:], ctxB[:])
                    for i in range(4):
                        pieces.append(piece(csA, sc, hA, i))
                        pieces.append(piece(csB, sc, hB, i))

            # drain remaining filler work
            while work:
                work.popleft()()
            while pieces:
                pieces.popleft()()

    nc.compile()
    return nc


def kernel(Q, Wq, bq, Wk, bk, Wv, bv):
    global _cached_nc, last_result
    Q = np.asarray(Q, dtype=np.float32)
    Wq, Wk, Wv = (np.asarray(w, dtype=np.float32) for w in (Wq, Wk, Wv))
    bq, bk, bv = (np.asarray(b, dtype=np.float32) for b in (bq, bk, bv))
    B = Q.shape[0]
    assert Q.shape == (B, S, D) and B * 4 == N_CORES

    if _cached_nc is None:
        _cached_nc = _build()
    nc = _cached_nc

    # host-side shard prep
    qts = [_round_fp32r(Q[b].T) for b in range(B)]
    wqs = [_round_fp32r(Wq[g * J : (g + 1) * J, :].T) for g in range(4)]
    wks = [_round_fp32r(Wk[g * J : (g + 1) * J, :].T) for g in range(4)]
    wvs = [_round_fp32r(Wv[g * J : (g + 1) * J, :].T) for g in range(4)]

    in_maps = []
    for c in range(N_CORES):
        b, g = c // 4, c % 4
        jsl = slice(g * J, (g + 1) * J)
        in_maps.append(
            {
                "qt": qts[b],
                "wq": wqs[g],
                "wk": wks[g],
                "wv": wvs[g],
                "bq": np.ascontiguousarray(bq[jsl]),
                "bk": np.ascontiguousarray(bk[jsl]),
                "bv": np.ascontiguousarray(bv[jsl]),
            }
        )

    last_result = run_bass_kernel_spmd(nc, in_maps, list(range(N_CORES)))

    full = np.empty((B, S, D), dtype=np.float32)
    for c in range(N_CORES):
        b, g = c // 4, c % 4
        full[b, :, g * J : (g + 1) * J] = last_result.results[c]["out"]
    return full
